# revision 15
# baseline (speedup 1.0000x reference)
"""CapsuleLayer (dynamic routing, N_IN=512, N_OUT=2, D=16, 3 iters) on 8 trn2
NeuronCores, pure data-parallel over the batch.

v5: dual-layout redesign.
- Layout A (partition=(ns,j), free=(g,b)) feeds the z-matmuls (row-tiled 4x32
  PE tiles, K=32), the uz elementwise product, and packed-compact dsum matmuls
  (col-tiled 128x32) whose output lands as delta[n mod 128, b] in natural n
  order per 128-block.
- Layout E (partition=n-in-block, free=(j,b)) feeds the P/t matmuls
  (col-tiled) and lets the softmax weight c[n,b] broadcast over j via a
  0-stride free-dim AP, so sigmoid runs COMPACT (4 ops/iter instead of 32).
- sigma: c0 = sigmoid(delta) computed per 128-n block directly from dsum PSUM.
- uw split DVE/GpSimd; z-evac on ACT paced at ~1.65us/3-group chunk.
- squash: |s|^2 via padded onesk matmul in the same (128,32) col mode.

softmax over k=2 == sigmoid of logit diff; squash(s) = g(|s|^2)*s.
"""

import numpy as np

N_CORES = 8
B = 4096
B_LOCAL = B // N_CORES          # 512
BLK = B_LOCAL
N_IN, N_OUT, D = 512, 2, 16
G = 64                          # groups of 8 capsules: 8*16 = 128 partitions
NB = 4                          # n-blocks of 128 capsules
KI = N_OUT * D                  # 32
EPS = 1e-07
ZR = 3                          # groups per z-round (3 => zq fits 2x3 banks)
NZROUND = (G + ZR - 1) // ZR    # 22 rounds (last has 1 group)
# of the 4 uw j-quads per block, this many go to GpSimd (rest DVE)
UW_GPS = 2

_CACHE = {}
DEBUG = None  # None | s1 | v0 | c0 | ssb1 | v1 | d0 | z0 | uz0
NO_LOOP_DMA = False   # timing-only: hoist ut DMAs out of the repeat loop
NO_SQRT = False       # timing-only: use Sigmoid instead of Sqrt in squash


# ---------------------------------------------------------------------------
# Walrus in this container allows only ONE sync-wait per TPB instruction.
# Tile attaches several sem waits to one instruction; split extras onto
# standalone NoOps (same engine, one wait each) inserted just before it.
# ---------------------------------------------------------------------------
def _apply_tile_patch():
    import concourse.tile as tile_mod
    from concourse import mybir
    from concourse.vector_clock import ScopedClock
    from concourse._compat import nn

    if getattr(tile_mod.TileContext, "_wait_split_patched", False):
        return

    _orig_add_instruction = tile_mod.TileContext._add_instruction

    def _split_waits(self, inst):
        si = inst.sync_info
        if si is None or len(si.on_wait) <= 1:
            return
        waits = list(si.on_wait)
        ups = list(si.on_update)
        inst.sync_info = mybir.SyncInfo(on_wait=[waits[-1]], on_update=ups)
        for i, w in enumerate(waits[:-1]):
            nop = mybir.InstNoOp(name=f"{inst.name}-wsplit{i}", ins=[], outs=[])
            nop.engine = inst.engine
            nop.sync_info = mybir.SyncInfo(on_wait=[w], on_update=[])
            self.nc.register_instruction(nop, overwrite=True)
            nn(self.nc.cur_bb).bb.add_instruction(nop)

    def _patched_add_instruction(self, inst):
        _split_waits(self, inst)
        _orig_add_instruction(self, inst)

    def _patched_drain_and_barrier(self, tick_clock, wait_clock):
        nc = self.nc
        drain_inst = nc.sync.drain()
        wait_clock.add_sem_waits(
            drain_inst.ins, ScopedClock({None: tick_clock.global_clock})
        )
        si = drain_inst.ins.sync_info
        if si is not None and len(si.on_wait) > 1:
            waits = list(si.on_wait)
            ups = list(si.on_update)
            drain_inst.ins.sync_info = mybir.SyncInfo(
                on_wait=[waits[0]], on_update=ups
            )
            for w in waits[1:]:
                nop = nc.sync.nop(nofuse=True)
                nop.ins.sync_info = mybir.SyncInfo(on_wait=[w], on_update=[])

        nc.all_engine_barrier()
        assert self.sems is not None
        popped = nc._tile_sem_poison_stack.pop()
        assert popped is self._sem_poison
        nc.clear_and_free_semaphores(list(self.sems.allocated().values()))
        nc.all_engine_barrier()

    tile_mod.TileContext._add_instruction = _patched_add_instruction
    tile_mod.TileContext._drain_and_barrier = _patched_drain_and_barrier
    tile_mod.TileContext._wait_split_patched = True


# ---------------------------------------------------------------------------
# Host-side constant prep from W  (W: [1, 512, 2, 16, 16] f32, idx [_,n,k,i,j])
# ---------------------------------------------------------------------------
def _prep_consts(W):
    import ml_dtypes

    bf16 = ml_dtypes.bfloat16
    W = np.asarray(W, dtype=np.float32).reshape(N_IN, N_OUT, D, D)  # [n,k,i,j]
    Wg = W.reshape(G, 8, N_OUT, D, D)                   # [g, ns, k, i, j]

    # z-MM lhsT, replicated at all 4 row strips:
    # zpadr[32s + (k,i), g, (ns,j)] = sign(k) * W[8g+ns, k, i, j]
    zfull = np.transpose(Wg, (0, 2, 3, 1, 4)).reshape(G, KI, 128).copy()
    zfull[:, D:, :] *= -1.0                             # k=1 rows negative
    zt = np.transpose(zfull, (1, 0, 2))                 # [32, G, 128]
    zpadr = np.concatenate([zt, zt, zt, zt], axis=0)    # [128, G, 128]

    # t/P-MM lhsT (layout E): we[p, j, blk, ki] = W[128*blk + p, k, i, j]
    we = np.transpose(W.reshape(N_IN, KI, D), (0, 2, 1))    # [n, j, ki]
    we = we.reshape(NB, 128, D, KI).transpose(1, 2, 0, 3)   # [128, j, blk, ki]

    # dsum lhsT variants: ones_r[(ns,j), r, c] = 1 iff c == 8r + ns
    ones_r = np.zeros((128, 4, 32), dtype=np.float32)
    for ns in range(8):
        for j in range(D):
            for r in range(4):
                ones_r[16 * ns + j, r, 8 * r + ns] = 1.0

    # squash norm lhsT padded to 128 contraction rows (rows 32: zero)
    oneskp = np.zeros((128, KI), dtype=np.float32)
    oneskp[:KI] = np.kron(np.eye(2, dtype=np.float32), np.ones((D, D), np.float32))

    km = np.concatenate([np.ones(D, np.float32), -np.ones(D, np.float32)])[:, None]
    pmsk = np.concatenate([np.zeros(D, np.float32), np.ones(D, np.float32)])[:, None]
    return {
        "zpadr": np.ascontiguousarray(zpadr.astype(bf16)),    # [128, 64, 128]
        "we": np.ascontiguousarray(we.astype(bf16)),          # [128, 16, 4, 32]
        "ones_r": np.ascontiguousarray(ones_r.astype(bf16)),  # [128, 4, 32]
        "oneskp": np.ascontiguousarray(oneskp.astype(bf16)),  # [128, 32]
        "kmask": np.ascontiguousarray(km),                    # [32, 1] f32
        "pmask": np.ascontiguousarray(pmsk),                  # [32, 1] f32
    }


def _prep_u(inputs):
    """Full inputs [B, 8,8,8,16] f32 -> per-core ut_a and ut_e bf16."""
    import ml_dtypes

    bf16 = ml_dtypes.bfloat16
    u = np.asarray(inputs, dtype=np.float32).reshape(B, N_IN * D)
    uas, ues = [], []
    for c in range(N_CORES):
        slab = u[c * B_LOCAL : (c + 1) * B_LOCAL].astype(bf16)  # [512, 8192]
        # layout A: [(ns,j)=128, g, b]
        ua = slab.reshape(B_LOCAL, G, 128).transpose(2, 1, 0)
        uas.append(np.ascontiguousarray(ua))
        # layout E: [n_in_block=128, blk, j, b]
        ue = slab.reshape(B_LOCAL, N_IN, D).transpose(1, 2, 0)  # [n, j, b]
        ue = ue.reshape(NB, 128, D, B_LOCAL).transpose(1, 0, 2, 3)
        ues.append(np.ascontiguousarray(ue))
    return uas, ues


def make_in_maps(inputs, W):
    consts = _prep_consts(W)
    uas, ues = _prep_u(inputs)
    in_maps = []
    for c in range(N_CORES):
        m = {"ut_a": uas[c], "ut_e": ues[c]}
        m.update(consts)
        in_maps.append(m)
    return in_maps


# ---------------------------------------------------------------------------
# Bass program
# ---------------------------------------------------------------------------
def _build_program(repeat=1):
    import contextlib

    import concourse.bass as bass
    import concourse.tile as tile
    from concourse import mybir

    _apply_tile_patch()
    f32 = mybir.dt.float32
    bf16 = mybir.dt.bfloat16

    nc = bass.Bass(trn_type="TRN2", target_bir_lowering=False)
    uta_in = nc.declare_dram_parameter("ut_a", [128, G, BLK], bf16, isOutput=False)
    ute_in = nc.declare_dram_parameter("ut_e", [128, NB, D, BLK], bf16, isOutput=False)
    zpadr_in = nc.declare_dram_parameter("zpadr", [128, G, 128], bf16, isOutput=False)
    we_in = nc.declare_dram_parameter("we", [128, D, NB, KI], bf16, isOutput=False)
    onesr_in = nc.declare_dram_parameter("ones_r", [128, 4, KI], bf16, isOutput=False)
    oneskp_in = nc.declare_dram_parameter("oneskp", [128, KI], bf16, isOutput=False)
    kmask_in = nc.declare_dram_parameter("kmask", [KI, 1], f32, isOutput=False)
    pmask_in = nc.declare_dram_parameter("pmask", [KI, 1], f32, isOutput=False)
    v_out = nc.declare_dram_parameter("v", [KI, B_LOCAL], f32, isOutput=True)
    dbg_out = (
        nc.declare_dram_parameter("dbg", [128, B_LOCAL], f32, isOutput=True)
        if DEBUG else None
    )

    Sig = mybir.ActivationFunctionType.Sigmoid
    Sqrt = mybir.ActivationFunctionType.Sqrt

    with tile.TileContext(nc) as tc:
        with contextlib.ExitStack() as ctx:
            consts = ctx.enter_context(tc.tile_pool(name="consts", bufs=1))
            ut_p = ctx.enter_context(tc.tile_pool(name="ut", bufs=1))
            zs_p = ctx.enter_context(tc.tile_pool(name="zs", bufs=2))
            uz_p = ctx.enter_context(tc.tile_pool(name="uz", bufs=6))
            ce_p = ctx.enter_context(tc.tile_pool(name="ce", bufs=2))
            uw_p = ctx.enter_context(tc.tile_pool(name="uw", bufs=2))
            sm_p = ctx.enter_context(tc.tile_pool(name="sm", bufs=1))
            vp_p = ctx.enter_context(tc.tile_pool(name="vp", bufs=2))
            zq_p = ctx.enter_context(tc.tile_pool(name="zq", bufs=2, space="PSUM"))
            da_p = ctx.enter_context(tc.tile_pool(name="da", bufs=1, space="PSUM"))
            ta_p = ctx.enter_context(tc.tile_pool(name="ta", bufs=1, space="PSUM"))

            # --- constants to SBUF (outside repeat loop)
            zpadr = consts.tile([128, G, 128], bf16)
            nc.sync.dma_start(out=zpadr, in_=zpadr_in[:, :, :])
            we = consts.tile([128, D, NB, KI], bf16)
            nc.sync.dma_start(out=we, in_=we_in[:, :, :, :])
            ones_r = consts.tile([128, 4, KI], bf16)
            nc.sync.dma_start(out=ones_r, in_=onesr_in[:, :, :])
            oneskp = consts.tile([128, KI], bf16)
            nc.sync.dma_start(out=oneskp, in_=oneskp_in[:, :])
            kmask = consts.tile([KI, 1], f32)
            nc.sync.dma_start(out=kmask, in_=kmask_in[:, :])
            pmask = consts.tile([KI, 1], f32)
            nc.sync.dma_start(out=pmask, in_=pmask_in[:, :])
            # squash s^2 staging: rows 32:128 stay zero forever
            s2t = consts.tile([128, BLK], bf16)
            nc.vector.memset(s2t, 0)
            # preload ACT tables: sigmoid set is the in-loop resident one
            scr = consts.tile([KI, 1], f32)
            nc.scalar.activation(scr, kmask, Sqrt)
            nc.scalar.activation(scr, kmask, Sig)

            rep_cm = tc.For_i(0, repeat, 1) if repeat > 1 else contextlib.nullcontext()

            hoisted = {}
            if NO_LOOP_DMA:
                ute_h = ut_p.tile([128, NB, D, BLK], bf16, tag="ute")
                for blk in range(NB):
                    nc.sync.dma_start(out=ute_h[:, blk, :, :], in_=ute_in[:, blk, :, :])
                uta_h = ut_p.tile([128, G, BLK], bf16, tag="uta")
                for q4 in range(4):
                    nc.sync.dma_start(
                        out=uta_h[:, 16 * q4 : 16 * (q4 + 1), :],
                        in_=uta_in[:, 16 * q4 : 16 * (q4 + 1), :],
                    )
                hoisted = {"ute": ute_h, "uta": uta_h}

            # --- helpers -----------------------------------------------------
            def col_mm(out_ap, lhsT_ap, rhs_ap, start, stop, strip):
                nc.tensor.matmul(
                    out_ap, lhsT_ap, rhs_ap, start=start, stop=stop,
                    tile_position=(0, 32 * strip),
                )

            def squash(s_sb, tag):
                """s_sb [KI, BLK] f32 -> vt f32 [KI, BLK] (uses sqrt table)."""
                nc.scalar.square(s2t[:KI, :], s_sb)
                nsqt = da_p.tile([128, BLK], f32, tag="da", name=f"nsq{tag}")
                nsq = nsqt[:KI, :]
                col_mm(nsq, oneskp, s2t, True, True, 0)
                sqr = sm_p.tile([KI, BLK], f32, tag="sqr", name=f"sqr{tag}")
                nc.scalar.activation(sqr, nsq, Sig if NO_SQRT else Sqrt)
                nc.vector.tensor_scalar_add(sqr, sqr, EPS)
                den = sm_p.tile([KI, BLK], f32, tag="den", name=f"den{tag}")
                nc.vector.scalar_tensor_tensor(
                    out=den, in0=nsq, scalar=1.0, in1=sqr,
                    op0=mybir.AluOpType.add, op1=mybir.AluOpType.mult,
                )
                nc.vector.reciprocal(den, den)
                gfac = sm_p.tile([KI, BLK], f32, tag="sqr", name=f"gfac{tag}")
                nc.vector.tensor_mul(gfac, nsq, den)
                vt = vp_p.tile([KI, BLK], f32, tag="vt", name=f"vt{tag}")
                nc.vector.tensor_mul(vt, s_sb, gfac)
                return vt

            def replicate_v(vt, tag):
                """vt [KI, BLK] f32 -> vrep [128, BLK] bf16 (4 row copies)."""
                vrep = vp_p.tile([128, BLK], bf16, tag="vrep", name=f"vrep{tag}")
                nc.vector.tensor_copy(out=vrep[:KI, :], in_=vt)
                for r in range(1, 4):
                    nc.vector.tensor_copy(
                        out=vrep[32 * r : 32 * r + 32, :], in_=vrep[:KI, :]
                    )
                return vrep

            def t_phase(rhs_for, tag):
                """64 col-tiled MMs accumulating strips 0/1 of a psum bank.

                rhs_for(blk, j) -> AP [128, BLK] bf16.
                Returns psum tile [128, BLK]: strip s holds partial t over
                (blk, j) with (4*blk + j//4) % 2 == s... (any balanced split).
                """
                acc = ta_p.tile([128, BLK], f32, tag="ta", name=f"tacc{tag}")
                cnt = 0
                for blk in range(NB):
                    for j in range(D):
                        col_mm(
                            acc[:KI, :],
                            we[:, j, blk, :],
                            rhs_for(blk, j),
                            cnt == 0,
                            cnt == NB * D - 1,
                            0,
                        )
                        cnt += 1
                return acc

            def assemble_s(acc, pmp, tag):
                """s = kmask*T + pmp, T read straight from the psum strip."""
                s_sb = sm_p.tile([KI, BLK], f32, tag="ssb", name=f"ssb{tag}")
                nc.vector.scalar_tensor_tensor(
                    out=s_sb, in0=acc[:KI, :], scalar=kmask, in1=pmp,
                    op0=mybir.AluOpType.mult, op1=mybir.AluOpType.add,
                )
                return s_sb

            with rep_cm:
                # --- input DMA (layout E first: P-phase needs it)
                if NO_LOOP_DMA:
                    ut_e = hoisted["ute"]
                    ut_a = hoisted["uta"]
                else:
                    ut_e = ut_p.tile([128, NB, D, BLK], bf16, tag="ute")
                    ut_a = ut_p.tile([128, G, BLK], bf16, tag="uta")
                    for q4 in range(4):
                        nc.sync.dma_start(
                            out=ut_e[:, q4, :, :], in_=ute_in[:, q4, :, :]
                        )
                        nc.sync.dma_start(
                            out=ut_a[:, 16 * q4 : 16 * (q4 + 1), :],
                            in_=uta_in[:, 16 * q4 : 16 * (q4 + 1), :],
                        )

                # --- P-phase: P[ki,b] = sum_{n,j} W u  (col-tiled, strips 0/1)
                dbg_done = [False]

                def dbg_tap(name, ap, pdim):
                    if DEBUG == name and not dbg_done[0]:
                        dbg_done[0] = True
                        dt = consts.tile([128, BLK], f32, name="dbgt")
                        if pdim < 128:
                            nc.vector.memset(dt, 0)
                        nc.vector.tensor_copy(out=dt[:pdim, :], in_=ap)
                        nc.sync.dma_start(out=dbg_out[:, :], in_=dt)

                pacc = t_phase(lambda blk, j: ut_e[:, blk, j, :], "P")
                pmp = sm_p.tile([KI, BLK], f32, tag="pmp")
                nc.vector.tensor_scalar_mul(pmp, pacc[:KI, :], pmask)
                s1 = sm_p.tile([KI, BLK], f32, tag="ssb", name="ssb_i0")
                nc.vector.tensor_scalar_mul(s1, pacc[:KI, :], 0.5)
                dbg_tap("s1", s1, KI)
                vt_prev = squash(s1, "i0")
                dbg_tap("v0", vt_prev, KI)
                vsum0 = vt_prev

                for it in range(2):
                    if it == 0:
                        vrep = replicate_v(vt_prev, "i0")
                    else:
                        vsum = sm_p.tile([KI, BLK], f32, tag="vsum")
                        nc.vector.tensor_add(vsum, vsum0, vt_prev)
                        vrep = replicate_v(vsum, "i1")

                    # ---- delta chain: z rounds (row-tiled) + evac + uz,
                    #      with dsum/sigma/uw/t col-work interleaved by block.
                    uz_tiles = {}      # round -> (tile, ngroups)
                    dacc = {}          # blk -> psum tile
                    c_e = {}           # blk -> sbuf bf16 [128, BLK]
                    uw_bufs = {}       # (blk, q) -> tile

                    def z_round(r):
                        g0 = ZR * r
                        ng = min(ZR, G - g0)
                        zq = zq_p.tile(
                            [128, ZR, BLK], f32, tag="zq", name=f"zq{it}_{r}"
                        )
                        for i in range(ng):
                            nc.tensor.matmul(
                                zq[:, i, :],
                                zpadr[32 * i : 32 * i + 32, g0 + i, :],
                                vrep[32 * i : 32 * i + 32, :],
                                start=True, stop=True,
                                tile_position=(32 * i, 0),
                            )
                        zs = zs_p.tile(
                            [128, ZR, BLK], bf16, tag="zs", name=f"zs{it}_{r}"
                        )
                        if r % 5 == 4:
                            nc.vector.tensor_copy(
                                out=zs[:, :ng, :], in_=zq[:, :ng, :])
                        else:
                            nc.scalar.copy(out=zs[:, :ng, :], in_=zq[:, :ng, :])
                        uz = uz_p.tile(
                            [128, ZR, BLK], bf16, tag="uz", name=f"uzt{it}_{r}"
                        )
                        nc.vector.tensor_mul(
                            uz[:, :ng, :], ut_a[:, g0 : g0 + ng, :], zs[:, :ng, :]
                        )
                        uz_tiles[r] = (uz, ng)
                        if it == 0 and r == 0:
                            dbg_tap("z0", zs[:, 0, :], 128)
                            dbg_tap("uz0", uz[:, 0, :], 128)

                    def dsum_block(blk):
                        da = da_p.tile(
                            [128, BLK], f32, tag="da", name=f"da{it}_{blk}"
                        )
                        dacc[blk] = da
                        # rr-major so consecutive MMs hit 4 different strips
                        for rr in range(4):
                            for s in range(4):
                                g = 16 * blk + 4 * s + rr
                                uz, _ = uz_tiles[g // ZR]
                                col_mm(
                                    da[32 * s : 32 * s + 32, :],
                                    ones_r[:, rr, :],
                                    uz[:, g % ZR, :],
                                    rr == 0,
                                    rr == 3,
                                    s,
                                )

                    def sigma_block(blk):
                        ce = ce_p.tile(
                            [128, BLK], bf16, tag="ce", name=f"ce{it}_{blk}"
                        )
                        if it == 0 and blk == 0:
                            dbg_tap("d0", dacc[blk], 128)
                        nc.scalar.activation(ce, dacc[blk], Sig)
                        c_e[blk] = ce
                        if it == 0 and blk == 0:
                            dbg_tap("c0", ce, 128)

                    def uw_quad(blk, q, gps):
                        """uw for j in [4q, 4q+4), one block, c broadcast."""
                        uw = uw_p.tile(
                            [128, 4, BLK], bf16, tag="uw", name=f"uw{it}_{blk}_{q}"
                        )
                        eng = nc.gpsimd if gps else nc.vector
                        eng.tensor_mul(
                            uw,
                            ut_e[:, blk, 4 * q : 4 * q + 4, :],
                            c_e[blk].unsqueeze(1).broadcast_to((128, 4, BLK)),
                        )
                        uw_bufs[(blk, q)] = uw

                    tacc = ta_p.tile([128, BLK], f32, tag="ta", name=f"tacc{it}")
                    tn = [0]

                    def t_quad(blk, q):
                        uw = uw_bufs.pop((blk, q))
                        for i in range(4):
                            j = 4 * q + i
                            col_mm(
                                tacc[:KI, :],
                                we[:, j, blk, :],
                                uw[:, i, :],
                                tn[0] == 0,
                                tn[0] == NB * D - 1,
                                0,
                            )
                            tn[0] += 1

                    # --- interleaved issue schedule -------------------------
                    # col-work for block B is issued after the z-round covering
                    # group 16B+15, with slack; uw/t quads staggered so PE
                    # never reaches a t-MM before its uw can be ready.
                    # GpSimd quads (slow) issue first, consumed last.
                    colwork = []  # (round_when, seq, item)
                    seq = 0
                    for bb in range(NB):
                        rdy = (16 * bb + 15) // ZR + 1
                        colwork.append((rdy, seq, ("dsum", bb))); seq += 1
                        colwork.append((rdy, seq, ("sigma", bb))); seq += 1
                        for q in range(4 - UW_GPS):
                            colwork.append((rdy + 1 + q, seq, ("uw", bb, q, False)))
                            seq += 1
                            colwork.append((rdy + 2 + q, seq, ("t", bb, q)))
                            seq += 1
                        for q in range(4 - UW_GPS, 4):
                            colwork.append((rdy + 3, seq, ("uw", bb, q, True)))
                            seq += 1
                            colwork.append(
                                (rdy + 5 + 2 * (q - (4 - UW_GPS)), seq,
                                 ("t", bb, q)))
                            seq += 1
                    colwork.sort(key=lambda x: (x[0], x[1]))

                    def do_item(item):
                        if item[0] == "dsum":
                            dsum_block(item[1])
                        elif item[0] == "sigma":
                            sigma_block(item[1])
                        elif item[0] == "uw":
                            uw_quad(item[1], item[2], item[3])
                        else:
                            t_quad(item[1], item[2])

                    ci = 0
                    for r in range(NZROUND):
                        z_round(r)
                        while ci < len(colwork) and colwork[ci][0] <= r:
                            do_item(colwork[ci][2])
                            ci += 1
                    while ci < len(colwork):
                        do_item(colwork[ci][2])
                        ci += 1

                    s_sb = assemble_s(tacc, pmp, f"i{it + 1}")
                    if it == 0:
                        dbg_tap("ssb1", s_sb, KI)
                    vt_prev = squash(s_sb, f"i{it + 1}")
                    if it == 0:
                        dbg_tap("v1", vt_prev, KI)

                nc.sync.dma_start(out=v_out[:, :], in_=vt_prev)

    return nc


def _get_program(repeat=1):
    key = ("nc", repeat)
    if key not in _CACHE:
        _CACHE[key] = _build_program(repeat)
    return _CACHE[key]


# ---------------------------------------------------------------------------
# Public entry: full inputs -> full output
# ---------------------------------------------------------------------------
def kernel(inputs, W):
    from concourse.bass_utils import run_bass_kernel_spmd

    in_maps = make_in_maps(inputs, W)
    nc = _get_program()
    res = run_bass_kernel_spmd(nc, in_maps, list(range(N_CORES)))
    outs = []
    for c in range(N_CORES):
        vt = res.results[c]["v"]                  # [KI, B_LOCAL]
        outs.append(vt.T)                         # [B_LOCAL, KI]
    v = np.concatenate(outs, axis=0)              # [B, 32]
    return np.ascontiguousarray(v.reshape(B, 1, N_OUT, D).astype(np.float32))
